# revision 32
# baseline (speedup 1.0000x reference)
"""Causal multi-head attention block (B=2, T=2048, C=1024, H=16) on 8 TRN2
NeuronCores.

Sharding: Megatron-style tensor parallel over heads for QKV+attention
(core r owns heads {2r, 2r+1} = feature rows [128r, 128r+128) of q/k/v),
then a token-sharded output projection: instead of AllGathering y
(7 MB inbound wire per core at ~60 GB/s bus ~= 117 us of CC-stream time),
the cores AllToAll y so core r ends up with y_full[:, its token slice]
(0.875 MB wire per core, ~13 us) and computes ALL 1024 output features
for its 256-token slice of each batch entry with the full Wo (same FLOPs
as the feature-sharded O-projection: contract is 1024 either way).

On-device everything is computed feature-major (transposed):

  qT/kT/vT [128, 4096] = W_shard @ x^T          (x^T passed from host)
  ST tile [128k, 512q] = kT_slice.T @ qT_slice  (contract d=64)
  causal mask: add a -1e9 strictly-lower-triangular matrix into the St
      PSUM accumulation group via matmul(ident, mneg) on diagonal blocks
  PT = exp(ST * 1/sqrt(d))                      (no max-subtraction: logits
                                                 are ~N(0,1), |S|max ~ 6)
  yT [65, 512] += [v | 1].T @ PT                (row 64 = softmax denom)
  yT_norm = yT[0:64] * partition_broadcast(recip(yT[64]))
  AllToAll per batch entry: blocks of [128 local feats, 256 tokens]
  po [128 out-feats, 256 tok] per om-chunk = Wo_full @ y_full_slice

Performance structure (derived from perfetto/HAM analysis of the
AllGather baseline, 305 us):
- The PE HAM clock-gate throttles to 4/8..13/16 whenever the matmul
  stream has bubbles, which doubles matmul time; the entire schedule is
  built to keep the PE queue dense: attention pairs are emitted with a
  1-deep software pipeline skew (S(p+1) is program-ordered before PV(p),
  so the exp(p) latency is covered by real work), and the projections for
  later token-chunks are chopped into 2-matmul micro-fillers popped one
  per attention pair, with a block flush at each q-chunk boundary
  (deadline: chunk tch is needed by the next q-chunk's S matmuls).
- Attention starts as soon as tch0 is projected (~10 us) instead of
  after all projections; x DMAs are issued in tch order so data arrives
  just ahead of its projection.
- The two AllToAlls fire per batch as soon as that batch's heads are
  evicted; the CC entry barrier (~56 us rendezvous) overlaps compute.
- O-projections run at the tail: po(b0) covers the A2A(b1) latency,
  keep-warm matmuls bridge the small remaining gap.

Inputs are bf16 (host-side cast); accumulation is f32 in PSUM; the
output shard is written bf16 and upcast to f32 on the host.
"""

import numpy as np
import ml_dtypes

import concourse.bacc as bacc
import concourse.mybir as mybir
import concourse.tile as tile
from concourse.bass_utils import run_bass_kernel_spmd
from concourse.masks import make_identity

N_CORES = 8
B, T, C, H = 2, 2048, 1024, 16
D = 64                # head dim
HL = H // N_CORES     # heads per core = 2
DL = HL * D           # local feature dim = 128
TT = B * T            # 4096 tokens total
P = 128
NCH = C // P          # 8 contraction chunks
QCH = 512             # projection token-chunk (moving free dim)
NQC = T // QCH        # 4 projection chunks per batch entry
WQ = 1024             # attention q-window (bf16 moving-operand max)
NWQ = T // WQ         # 2 q-windows per batch entry
NKT = T // P          # 16 k-tiles per batch entry
TSH = T // N_CORES    # 256: tokens per core per batch after AllToAll
SCALE = 1.0 / np.sqrt(D)

BF = mybir.dt.bfloat16
F32 = mybir.dt.float32
AF = mybir.ActivationFunctionType

WQKV = 3 * NCH * DL           # 3072 cols of packed q/k/v shards
WO = NCH * NCH * P            # 8192 cols of packed full Wo
WCOLS = WQKV + WO             # 11264


def build_graph():
    nc = bacc.Bacc("TRN2", target_bir_lowering=False, debug=False)

    # [p, ci, t] with c = ci*128 + p: one contiguous DMA per token chunk
    xT = nc.dram_tensor("xT", [P, NCH * TT], BF, kind="ExternalInput")
    # q/k/v shards [p, w, ci, m] + full Wo [p, om, ci, m], pre-packed
    # host-side into SBUF layout: contiguous rows, loaded as 8 DMAs
    wall = nc.dram_tensor("wall", [P, WCOLS], BF, kind="ExternalInput")
    # (m, om, b, t): feature om*128+m, batch b, token-in-shard t
    out = nc.dram_tensor("out", [P, NCH, B, TSH], BF, kind="ExternalOutput")

    with tile.TileContext(nc) as tc:
        with (
            tc.tile_pool(name="sb", bufs=1) as sb,
            tc.tile_pool(name="ps", bufs=1, space="PSUM") as ps,
            tc.tile_pool(name="dram", bufs=1, space="DRAM") as dram,
        ):
            # ---- weight + constant loads ----
            w_sb = sb.tile([P, WCOLS], BF, name="w_sb")
            for pc in range(8):
                csl = slice(pc * (WCOLS // 8), (pc + 1) * (WCOLS // 8))
                nc.sync.dma_start(w_sb[:, csl], wall[:, csl])
            w3 = w_sb[:, 0:WQKV].rearrange("p (w a m) -> p w a m", w=3, a=NCH)
            wq_sb, wk_sb, wv_sb = (w3[:, i] for i in range(3))
            wo_sb = w_sb[:, WQKV:WCOLS].rearrange(
                "p (o a m) -> p o a m", o=NCH, a=NCH
            )

            ident = sb.tile([P, P], BF, name="ident")
            make_identity(nc, ident)
            wsrc = sb.tile([P, QCH], BF, name="wsrc")
            nc.vector.memset(wsrc[:], 0.5)

            xT_sb = sb.tile([P, NCH, TT], BF, name="xT_sb")
            xTv = xT[:].rearrange("p (a t) -> p a t", a=NCH)
            # x lands in tch order just ahead of each chunk's projection;
            # gpsimd (SWDGE) so the scalar queue stays free for the exps
            # and sync for the weights
            nc.gpsimd.dma_start(xT_sb[:, :, 0:QCH], xTv[:, :, 0:QCH])
            nc.gpsimd.dma_start(xT_sb[:, :, QCH:2 * QCH],
                                xTv[:, :, QCH:2 * QCH])

            # strictly-lower-triangular -1e9 (k > q): masks causal logits on
            # diagonal blocks, injected into the St PSUM group via
            # matmul(ident, mneg)
            mneg = sb.tile([P, P], BF, name="mneg")
            nc.gpsimd.memset(mneg[:], 0.0)
            nc.gpsimd.affine_select(
                out=mneg[:], in_=mneg[:],
                compare_op=mybir.AluOpType.is_ge,
                fill=-1e9, base=0, channel_multiplier=-1, pattern=[[1, P]],
            )

            def warm(n):
                for _ in range(n):
                    wdst = ps.tile([P, QCH], F32, tag="fx", bufs=2,
                                   name="wdst")
                    nc.tensor.matmul(wdst[:], ident[:], wsrc[:],
                                     start=True, stop=True)

            qT_sb = sb.tile([P, TT], BF, name="qT_sb")
            kT_sb = sb.tile([P, TT], BF, name="kT_sb")
            vT_sb = sb.tile([P, TT], BF, name="vT_sb")
            # v in natural layout, packed per 128-token tile as
            # [headA(64) | 1 | headB(64) | 1] -> 130 columns
            v_sb = sb.tile([P, TT // P, 2 * (D + 1)], BF, name="v_sb")
            nc.gpsimd.memset(v_sb[:], 1.0)
            # remaining x chunks after the memset (needed later anyway)
            for tch in range(2, 8):
                tsl = slice(tch * QCH, (tch + 1) * QCH)
                nc.gpsimd.dma_start(xT_sb[:, :, tsl], xTv[:, :, tsl])

            # AllToAll buffers: per batch, 8 blocks of [128 feats, 256 tok]
            ain = [dram.tile([N_CORES, DL, TSH], BF, name=f"ain{b}")
                   for b in range(B)]
            aout = [dram.tile([N_CORES, DL, TSH], BF, name=f"aout{b}")
                    for b in range(B)]

            # ---- projection micro-fillers (cost, closure) pairs; cost is
            # PE cycles so the attention pacer can fill each unit's slack
            def make_proj_fillers(tch):
                """2-matmul closures for q/k/v projection of token chunk
                tch + the 4 v-transposes."""
                tsl = slice(tch * QCH, (tch + 1) * QCH)
                fillers = []
                for wsb, dst in ((wq_sb, qT_sb), (wk_sb, kT_sb),
                                 (wv_sb, vT_sb)):
                    cell = {}
                    def mk(ci0, wsb=wsb, dst=dst, cell=cell):
                        def f():
                            if ci0 == 0:
                                cell["pj"] = ps.tile([P, QCH], F32,
                                                     tag="fx", bufs=2,
                                                     name="pj")
                            pj = cell["pj"]
                            for ci in (ci0, ci0 + 1):
                                nc.tensor.matmul(
                                    pj[:], wsb[:, ci, :], xT_sb[:, ci, tsl],
                                    start=(ci == 0), stop=(ci == NCH - 1),
                                )
                            if ci0 == NCH - 2:
                                nc.vector.tensor_copy(dst[:, tsl], pj[:])
                        return f
                    for ci0 in range(0, NCH, 2):
                        fillers.append((1024, mk(ci0)))
                for t32 in range(tch * 4, tch * 4 + 4):
                    def vt(t32=t32):
                        tr = ps.tile([P, P], BF, tag="fx", bufs=2,
                                     name="tr")
                        nc.tensor.transpose(
                            tr[:], vT_sb[:, t32 * P:(t32 + 1) * P], ident[:]
                        )
                        out_ap = v_sb[:, t32, :].rearrange(
                            "p (h x) -> p h x", h=HL
                        )[:, :, 0:D]
                        in_ap = tr[:].rearrange("p (h x) -> p h x", h=HL)
                        nc.vector.tensor_copy(out_ap, in_ap)
                    fillers.append((128, vt))
                return fillers

            # ---- attention emission machinery ----
            filler_q = []
            pending = []   # at most one (pv_fn, post_fn)

            def flush_pending():
                while pending:
                    pv, post = pending.pop(0)
                    pv()
                    if post:
                        post()

            def emit_unit(s_fn, pv_fn=None, post_fn=None, budget=0):
                """budget: PE cycles of filler to interleave so the unit's
                total PE work matches the ACT (exp) period — keeps the PE
                saturated (HAM at full clock) without delaying the exps."""
                s_fn()
                while budget > 0:
                    if filler_q:
                        cost, f = filler_q.pop(0)
                        f()
                        budget -= cost
                    else:
                        # dummy matmul: keeps the PE above the HAM
                        # activity threshold when no real filler remains
                        warm_filler()
                        budget -= 512
                if pending:
                    pv, post = pending.pop(0)
                    pv()
                    if post:
                        post()
                if pv_fn:
                    pending.append((pv_fn, post_fn))

            def flush_fillers():
                while filler_q:
                    filler_q.pop(0)[1]()

            def mk_evict(b, jq, h, hstate, then=None):
                def f():
                    yt, den = hstate["yt"], hstate["den"]
                    bc = sb.tile([D, QCH], F32, tag="bc", bufs=3, name="bc")
                    nc.gpsimd.partition_broadcast(bc[:], den[:])
                    rcp = sb.tile([D, QCH], F32, tag="rcp", bufs=3,
                                  name="rcp")
                    nc.vector.reciprocal_approx_fast(rcp[:], bc[:])
                    yn = sb.tile([D, QCH], BF, tag="yn", bufs=4, name="yn")
                    nc.vector.tensor_mul(yn[:], yt[0:D, :], rcp[:])
                    for s in range(2):
                        nc.gpsimd.dma_start(
                            ain[b][2 * jq + s, h * D:(h + 1) * D, :],
                            yn[:, s * TSH:(s + 1) * TSH],
                        )
                    if then:
                        then()
                return f

            def emit_head(b, jq, h, then=None):
                rsl = slice(h * D, (h + 1) * D)
                q0 = b * T + jq * QCH
                nkt = 4 * jq + 4
                hstate = {}
                cells = [dict() for _ in range(nkt // 2)]

                def mk_s(pr):
                    def f():
                        if pr == 0:
                            hstate["yt"] = ps.tile([D + 1, QCH], F32,
                                                   tag="yt", bufs=2,
                                                   name="yt")
                            hstate["den"] = sb.tile([1, QCH], F32,
                                                    tag="den", bufs=4,
                                                    name="den")
                        st = ps.tile([P, 2 * QCH], F32, tag="st", bufs=2,
                                     name="st")
                        pt = sb.tile([P, 2 * QCH], BF, tag="pt", bufs=4,
                                     name="pt")
                        cells[pr]["pt"] = pt
                        masks = []
                        for half in range(2):
                            kt = 2 * pr + half
                            k0 = b * T + kt * P
                            i = kt - 4 * jq
                            qv = max(i, 0) * P
                            ssl = slice(half * QCH + qv, (half + 1) * QCH)
                            nc.tensor.matmul(
                                st[:, ssl],
                                kT_sb[rsl, k0:k0 + P],
                                qT_sb[rsl, q0 + qv:q0 + QCH],
                                start=True, stop=(i < 0),
                            )
                            if i >= 0:
                                masks.append(half * QCH + qv)
                        # mask matmuls after both S halves: ident stays
                        # loaded as the stationary across consecutive ones
                        for m0 in masks:
                            nc.tensor.matmul(
                                st[:, m0:m0 + P], ident[:], mneg[:],
                                start=False, stop=True,
                            )
                        qv0 = max(2 * pr - 4 * jq, 0) * P
                        nc.scalar.activation(
                            pt[:, qv0:], st[:, qv0:], AF.Exp,
                            scale=float(SCALE)
                        )
                    return f

                def mk_pv(pr):
                    def f():
                        pt = cells[pr]["pt"]
                        yt = hstate["yt"]
                        for half in range(2):
                            kt = 2 * pr + half
                            qv = max(kt - 4 * jq, 0) * P
                            nc.tensor.matmul(
                                yt[:, qv:QCH],
                                v_sb[:, b * NKT + kt,
                                     h * (D + 1):(h + 1) * (D + 1)],
                                pt[:, half * QCH + qv:(half + 1) * QCH],
                                start=(kt == 0), stop=(kt == nkt - 1),
                            )
                        if pr == nkt // 2 - 1:
                            nc.vector.tensor_copy(hstate["den"][:],
                                                  yt[D:D + 1, :])
                    return f

                last = nkt // 2 - 1
                for pr in range(nkt // 2):
                    qv0 = max(2 * pr - 4 * jq, 0) * P
                    qv1 = max(2 * pr + 1 - 4 * jq, 0) * P
                    real = 2 * ((QCH - qv0) + (QCH - qv1))
                    real += 128 * ((2 * pr >= 4 * jq) + (2 * pr + 1 >= 4 * jq))
                    # exp period in PE-clock cycles: ACT runs at half the
                    # PE clock; ~490 covers the 352-cycle fixed cost plus
                    # observed PSUM-read contention
                    act = 2 * (2 * QCH - qv0 + 490)
                    emit_unit(
                        mk_s(pr), mk_pv(pr),
                        mk_evict(b, jq, h, hstate, then) if pr == last
                        else None,
                        budget=act - real,
                    )

            def a2a_fire(b):
                nc.gpsimd.collective_compute(
                    "AllToAll",
                    mybir.AluOpType.bypass,
                    replica_groups=[list(range(N_CORES))],
                    ins=[ain[b][:]],
                    outs=[aout[b][:]],
                )

            yf_tiles = {}

            def yf_load(b):
                yf = sb.tile([P, NCH, TSH], BF, tag="yf", bufs=2,
                             name="yf")
                yf_tiles[b] = yf
                nc.sync.dma_start(
                    yf[:, :, :],
                    aout[b][:].rearrange("a p t -> p a t"),
                )

            def po_group(b, store_engine):
                yf = yf_tiles[b]
                for om in range(NCH):
                    po = ps.tile([P, TSH], F32, tag="fx", bufs=2,
                                 name="po")
                    for ci in range(NCH):
                        nc.tensor.matmul(
                            po[:], wo_sb[:, om, ci, :], yf[:, ci, :],
                            start=(ci == 0), stop=(ci == NCH - 1),
                        )
                    ob = sb.tile([P, TSH], BF, tag="ob", bufs=2, name="ob")
                    nc.vector.tensor_copy(ob[:], po[:])
                    store_engine.dma_start(out[:, om, b, :], ob[:])

            # ---- prologue: warmups cover the first x DMAs ----
            warm(8)
            # tch0 projections as a dense block, then attention starts
            filler_q.extend(make_proj_fillers(0))
            flush_fillers()

            # filler deadline map: during (b, jq) we drip-feed the chunk
            # needed by the NEXT q-chunk phase, flushing at the boundary
            next_tch = {(0, 0): 1, (0, 1): 2, (0, 2): 3, (0, 3): 4,
                        (1, 0): 5, (1, 1): 6, (1, 2): 7}

            def warm_filler():
                wdst = ps.tile([P, QCH], F32, tag="fx", bufs=2, name="wdst")
                nc.tensor.matmul(wdst[:], ident[:], wsrc[:],
                                 start=True, stop=True)

            for b in range(B):
                for jq in range(NQC):
                    if b == 1 and jq == 0:
                        # cover the batch-boundary eviction flush so the
                        # PE stays dense while the vector chain drains
                        filler_q.extend([(512, warm_filler)] * 6)
                    if (b, jq) in next_tch:
                        filler_q.extend(make_proj_fillers(next_tch[(b, jq)]))
                    last_head = (jq == NQC - 1)
                    for h in range(HL):
                        # fire the batch's collective right after its last
                        # eviction DMAs; yf(1) SBUF loads are deferred to
                        # the tail so their DMA-lane sem increments don't
                        # precede po(0)'s stores in the scheduler's lane
                        # bookkeeping (po(0) would transitively wait on
                        # the second AllToAll)
                        then = None
                        if last_head and h == HL - 1:
                            if b == 0:
                                then = lambda: (a2a_fire(0), yf_load(0))
                            else:
                                then = lambda: a2a_fire(1)
                        emit_head(b, jq, h, then)
                    flush_fillers()
            # the final PV + evict + A2A(b1) are still pending here
            flush_pending()

            # ---- tail: po(b0) covers A2A(b1) latency; pinned late so the
            # scheduler cannot hoist the yf-gated matmuls into the
            # attention stream (they would stall the in-order PE queue)
            with tc.tile_wait_until(0.28):
                po_group(0, nc.scalar)
            with tc.tile_wait_until(0.285):
                yf_load(1)
                warm(24)
            with tc.tile_wait_until(0.3):
                po_group(1, nc.sync)

    nc.finalize()
    return nc


_GRAPH = None


def _get_graph():
    global _GRAPH
    if _GRAPH is None:
        _GRAPH = build_graph()
    return _GRAPH


def prepare_in_maps(x, Wq, Wk, Wv, Wo):
    x = np.asarray(x, np.float32)
    Wq = np.asarray(Wq, np.float32)
    Wk = np.asarray(Wk, np.float32)
    Wv = np.asarray(Wv, np.float32)
    Wo = np.asarray(Wo, np.float32)

    bf = ml_dtypes.bfloat16
    # [p, ci, t] with c = ci*128 + p
    xTh = np.ascontiguousarray(
        x.reshape(TT, NCH, P).transpose(2, 1, 0).reshape(P, NCH * TT)
    ).astype(bf)
    # full Wo packed [p, om, ci, m]: wo[p, om, ci, m] = Wo[om*128+m,
    # ci*128+p] (shared by all cores)
    woall = Wo.T.reshape(NCH, P, NCH, P).transpose(1, 2, 0, 3)
    woall = np.ascontiguousarray(woall.reshape(P, WO)).astype(bf)
    in_maps = []
    for r in range(N_CORES):
        sl = slice(r * DL, (r + 1) * DL)
        # pack the 3 transposed q/k/v shards into the SBUF layout
        # [p, w, ci, m] where the shard row index is c = ci*128 + p
        wqkv = np.empty((P, 3, NCH, DL), np.float32)
        for w, W in enumerate((Wq, Wk, Wv)):
            wqkv[:, w] = W[sl].T.reshape(NCH, P, DL).transpose(1, 0, 2)
        wall = np.concatenate(
            [np.ascontiguousarray(wqkv.reshape(P, WQKV)).astype(bf), woall],
            axis=1,
        )
        in_maps.append({
            "xT": xTh,
            "wall": np.ascontiguousarray(wall),
        })
    return in_maps


def assemble_output(results):
    outT = np.empty((B, C, T), np.float32)
    for r in range(N_CORES):
        o = np.asarray(results[r]["out"], np.float32)  # [m, om, b, t]
        # feature index = om*128 + m; token = r*256 + t within batch
        oT = o.transpose(2, 1, 0, 3).reshape(B, C, TSH)
        outT[:, :, r * TSH:(r + 1) * TSH] = oT
    return np.ascontiguousarray(outT.transpose(0, 2, 1))


def kernel(x, Wq, Wk, Wv, Wo):
    nc = _get_graph()
    in_maps = prepare_in_maps(x, Wq, Wk, Wv, Wo)
    res = run_bass_kernel_spmd(nc, in_maps, core_ids=list(range(N_CORES)))
    return assemble_output(res.results)


# revision 33
# speedup vs baseline: 1.3142x; 1.3142x over previous
"""Causal multi-head attention block (B=2, T=2048, C=1024, H=16) on 8 TRN2
NeuronCores.

Sharding: Megatron-style tensor parallel over heads for QKV+attention
(core r owns heads {2r, 2r+1} = feature rows [128r, 128r+128) of q/k/v),
then a token-sharded output projection: instead of AllGathering y
(7 MB inbound wire per core at ~60 GB/s bus ~= 117 us of CC-stream time),
the cores AllToAll y so core r ends up with y_full[:, its token slice]
(0.875 MB wire per core, ~13 us) and computes ALL 1024 output features
for its 256-token slice of each batch entry with the full Wo (same FLOPs
as the feature-sharded O-projection: contract is 1024 either way).

On-device everything is computed feature-major (transposed):

  qT/kT/vT [128, 4096] = W_shard @ x^T          (x^T passed from host)
  ST tile [128k, 512q] = kT_slice.T @ qT_slice  (contract d=64)
  causal mask: add a -1e9 strictly-lower-triangular matrix into the St
      PSUM accumulation group via matmul(ident, mneg) on diagonal blocks
  PT = exp(ST * 1/sqrt(d))                      (no max-subtraction: logits
                                                 are ~N(0,1), |S|max ~ 6)
  yT [65, 512] += [v | 1].T @ PT                (row 64 = softmax denom)
  yT_norm = yT[0:64] * partition_broadcast(recip(yT[64]))
  AllToAll per batch entry: blocks of [128 local feats, 256 tokens]
  po [128 out-feats, 256 tok] per om-chunk = Wo_full @ y_full_slice

Performance structure (derived from perfetto/HAM analysis; the AllGather
baseline ran 305 us, this runs ~230 us true path):
- The PE HAM clock-gate is a proportional governor (K tracks array
  activity), so the schedule keeps the PE queue saturated: attention
  pairs are emitted with a 1-deep software pipeline skew (S(p+1) is
  program-ordered before PV(p), covering the exp(p) latency with real
  work), and each pair unit pops projection micro-fillers (2-matmul
  closures) sized by a cycle budget = exp-period minus the unit's own
  S/PV work — the PE stays dense without delaying the ACT-bound exp
  cadence; dummy 512-col matmuls fill when no real filler remains.
- Attention starts as soon as tch0 is projected; x arrives via one
  contiguous DMA per token chunk ([p, ci, t] host layout) on the gpsimd
  queue so the scalar queue is free for the exps (a DMA issue occupies
  its issuing engine queue ~0.7 us).
- The two AllToAlls fire per batch as soon as that batch's heads are
  evicted; the CC entry barrier (launch-skew rendezvous) overlaps
  compute. DMA-completion semaphore lanes are shared round-robin across
  queues, so yf(1)'s loads are program-ordered AFTER po(0)'s stores —
  otherwise po(0)'s release thresholds would transitively wait on the
  second collective.
- O-projections run at the tail, pinned there with tile_wait_until so
  the scheduler cannot hoist the collective-gated matmuls into the
  attention stream (the in-order PE queue would stall); po(b0) covers
  the A2A(b1) latency, keep-warm matmuls bridge the remaining gap.

Inputs are bf16 (host-side cast); accumulation is f32 in PSUM; the
output shard is written bf16 and upcast to f32 on the host.
"""

import numpy as np
import ml_dtypes

import concourse.bacc as bacc
import concourse.mybir as mybir
import concourse.tile as tile
from concourse.bass_utils import run_bass_kernel_spmd
from concourse.masks import make_identity

N_CORES = 8
B, T, C, H = 2, 2048, 1024, 16
D = 64                # head dim
HL = H // N_CORES     # heads per core = 2
DL = HL * D           # local feature dim = 128
TT = B * T            # 4096 tokens total
P = 128
NCH = C // P          # 8 contraction chunks
QCH = 512             # projection token-chunk (moving free dim)
NQC = T // QCH        # 4 projection chunks per batch entry
WQ = 1024             # attention q-window (bf16 moving-operand max)
NWQ = T // WQ         # 2 q-windows per batch entry
NKT = T // P          # 16 k-tiles per batch entry
TSH = T // N_CORES    # 256: tokens per core per batch after AllToAll
SCALE = 1.0 / np.sqrt(D)

BF = mybir.dt.bfloat16
F32 = mybir.dt.float32
AF = mybir.ActivationFunctionType

WQKV = 3 * NCH * DL           # 3072 cols of packed q/k/v shards
WO = NCH * NCH * P            # 8192 cols of packed full Wo
WCOLS = WQKV + WO             # 11264


def build_graph():
    nc = bacc.Bacc("TRN2", target_bir_lowering=False, debug=False)

    # [p, ci, t] with c = ci*128 + p: one contiguous DMA per token chunk
    xT = nc.dram_tensor("xT", [P, NCH * TT], BF, kind="ExternalInput")
    # q/k/v shards [p, w, ci, m] + full Wo [p, om, ci, m], pre-packed
    # host-side into SBUF layout: contiguous rows, loaded as 8 DMAs
    wall = nc.dram_tensor("wall", [P, WCOLS], BF, kind="ExternalInput")
    # (m, om, b, t): feature om*128+m, batch b, token-in-shard t
    out = nc.dram_tensor("out", [P, NCH, B, TSH], BF, kind="ExternalOutput")

    with tile.TileContext(nc) as tc:
        with (
            tc.tile_pool(name="sb", bufs=1) as sb,
            tc.tile_pool(name="ps", bufs=1, space="PSUM") as ps,
            tc.tile_pool(name="dram", bufs=1, space="DRAM") as dram,
        ):
            # ---- weight + constant loads ----
            w_sb = sb.tile([P, WCOLS], BF, name="w_sb")
            for pc in range(8):
                csl = slice(pc * (WCOLS // 8), (pc + 1) * (WCOLS // 8))
                nc.sync.dma_start(w_sb[:, csl], wall[:, csl])
            w3 = w_sb[:, 0:WQKV].rearrange("p (w a m) -> p w a m", w=3, a=NCH)
            wq_sb, wk_sb, wv_sb = (w3[:, i] for i in range(3))
            wo_sb = w_sb[:, WQKV:WCOLS].rearrange(
                "p (o a m) -> p o a m", o=NCH, a=NCH
            )

            ident = sb.tile([P, P], BF, name="ident")
            make_identity(nc, ident)
            wsrc = sb.tile([P, QCH], BF, name="wsrc")
            nc.vector.memset(wsrc[:], 0.5)

            xT_sb = sb.tile([P, NCH, TT], BF, name="xT_sb")
            xTv = xT[:].rearrange("p (a t) -> p a t", a=NCH)
            # x lands in tch order just ahead of each chunk's projection;
            # gpsimd (SWDGE) so the scalar queue stays free for the exps
            # and sync for the weights
            nc.gpsimd.dma_start(xT_sb[:, :, 0:QCH], xTv[:, :, 0:QCH])
            nc.gpsimd.dma_start(xT_sb[:, :, QCH:2 * QCH],
                                xTv[:, :, QCH:2 * QCH])

            # strictly-lower-triangular -1e9 (k > q): masks causal logits on
            # diagonal blocks, injected into the St PSUM group via
            # matmul(ident, mneg)
            mneg = sb.tile([P, P], BF, name="mneg")
            nc.gpsimd.memset(mneg[:], 0.0)
            nc.gpsimd.affine_select(
                out=mneg[:], in_=mneg[:],
                compare_op=mybir.AluOpType.is_ge,
                fill=-1e9, base=0, channel_multiplier=-1, pattern=[[1, P]],
            )

            def warm(n):
                for _ in range(n):
                    wdst = ps.tile([P, QCH], F32, tag="fx", bufs=2,
                                   name="wdst")
                    nc.tensor.matmul(wdst[:], ident[:], wsrc[:],
                                     start=True, stop=True)

            qT_sb = sb.tile([P, TT], BF, name="qT_sb")
            kT_sb = sb.tile([P, TT], BF, name="kT_sb")
            vT_sb = sb.tile([P, TT], BF, name="vT_sb")
            # v in natural layout, packed per 128-token tile as
            # [headA(64) | 1 | headB(64) | 1] -> 130 columns
            v_sb = sb.tile([P, TT // P, 2 * (D + 1)], BF, name="v_sb")
            nc.gpsimd.memset(v_sb[:], 1.0)
            # remaining x chunks after the memset (needed later anyway)
            for tch in range(2, 8):
                tsl = slice(tch * QCH, (tch + 1) * QCH)
                nc.gpsimd.dma_start(xT_sb[:, :, tsl], xTv[:, :, tsl])

            # AllToAll buffers: per batch, 8 blocks of [128 feats, 256 tok]
            ain = [dram.tile([N_CORES, DL, TSH], BF, name=f"ain{b}")
                   for b in range(B)]
            aout = [dram.tile([N_CORES, DL, TSH], BF, name=f"aout{b}")
                    for b in range(B)]

            # ---- projection micro-fillers (cost, closure) pairs; cost is
            # PE cycles so the attention pacer can fill each unit's slack
            def make_proj_fillers(tch):
                """2-matmul closures for q/k/v projection of token chunk
                tch + the 4 v-transposes."""
                tsl = slice(tch * QCH, (tch + 1) * QCH)
                fillers = []
                for wsb, dst in ((wq_sb, qT_sb), (wk_sb, kT_sb),
                                 (wv_sb, vT_sb)):
                    cell = {}
                    def mk(ci0, wsb=wsb, dst=dst, cell=cell):
                        def f():
                            if ci0 == 0:
                                cell["pj"] = ps.tile([P, QCH], F32,
                                                     tag="fx", bufs=2,
                                                     name="pj")
                            pj = cell["pj"]
                            for ci in (ci0, ci0 + 1):
                                nc.tensor.matmul(
                                    pj[:], wsb[:, ci, :], xT_sb[:, ci, tsl],
                                    start=(ci == 0), stop=(ci == NCH - 1),
                                )
                            if ci0 == NCH - 2:
                                nc.vector.tensor_copy(dst[:, tsl], pj[:])
                        return f
                    for ci0 in range(0, NCH, 2):
                        fillers.append((1024, mk(ci0)))
                for t32 in range(tch * 4, tch * 4 + 4):
                    def vt(t32=t32):
                        tr = ps.tile([P, P], BF, tag="fx", bufs=2,
                                     name="tr")
                        nc.tensor.transpose(
                            tr[:], vT_sb[:, t32 * P:(t32 + 1) * P], ident[:]
                        )
                        out_ap = v_sb[:, t32, :].rearrange(
                            "p (h x) -> p h x", h=HL
                        )[:, :, 0:D]
                        in_ap = tr[:].rearrange("p (h x) -> p h x", h=HL)
                        nc.vector.tensor_copy(out_ap, in_ap)
                    fillers.append((128, vt))
                return fillers

            # ---- attention emission machinery ----
            filler_q = []
            pending = []   # at most one (pv_fn, post_fn)

            def flush_pending():
                while pending:
                    pv, post = pending.pop(0)
                    pv()
                    if post:
                        post()

            def emit_unit(s_fn, pv_fn=None, post_fn=None, budget=0):
                """budget: PE cycles of filler to interleave so the unit's
                total PE work matches the ACT (exp) period — keeps the PE
                saturated (HAM at full clock) without delaying the exps."""
                s_fn()
                while budget > 0:
                    if filler_q:
                        cost, f = filler_q.pop(0)
                        f()
                        budget -= cost
                    else:
                        # dummy matmul: keeps the PE above the HAM
                        # activity threshold when no real filler remains
                        warm_filler()
                        budget -= 512
                if pending:
                    pv, post = pending.pop(0)
                    pv()
                    if post:
                        post()
                if pv_fn:
                    pending.append((pv_fn, post_fn))

            def flush_fillers():
                while filler_q:
                    filler_q.pop(0)[1]()

            def mk_evict(b, jq, h, hstate, then=None):
                def f():
                    yt, den = hstate["yt"], hstate["den"]
                    bc = sb.tile([D, QCH], F32, tag="bc", bufs=3, name="bc")
                    nc.gpsimd.partition_broadcast(bc[:], den[:])
                    rcp = sb.tile([D, QCH], F32, tag="rcp", bufs=3,
                                  name="rcp")
                    nc.vector.reciprocal_approx_fast(rcp[:], bc[:])
                    yn = sb.tile([D, QCH], BF, tag="yn", bufs=4, name="yn")
                    nc.vector.tensor_mul(yn[:], yt[0:D, :], rcp[:])
                    for s in range(2):
                        nc.gpsimd.dma_start(
                            ain[b][2 * jq + s, h * D:(h + 1) * D, :],
                            yn[:, s * TSH:(s + 1) * TSH],
                        )
                    if then:
                        then()
                return f

            def emit_head(b, jq, h, then=None):
                rsl = slice(h * D, (h + 1) * D)
                q0 = b * T + jq * QCH
                nkt = 4 * jq + 4
                hstate = {}
                cells = [dict() for _ in range(nkt // 2)]

                def mk_s(pr):
                    def f():
                        if pr == 0:
                            hstate["yt"] = ps.tile([D + 1, QCH], F32,
                                                   tag="yt", bufs=2,
                                                   name="yt")
                            hstate["den"] = sb.tile([1, QCH], F32,
                                                    tag="den", bufs=4,
                                                    name="den")
                        st = ps.tile([P, 2 * QCH], F32, tag="st", bufs=2,
                                     name="st")
                        pt = sb.tile([P, 2 * QCH], BF, tag="pt", bufs=4,
                                     name="pt")
                        cells[pr]["pt"] = pt
                        masks = []
                        for half in range(2):
                            kt = 2 * pr + half
                            k0 = b * T + kt * P
                            i = kt - 4 * jq
                            qv = max(i, 0) * P
                            ssl = slice(half * QCH + qv, (half + 1) * QCH)
                            nc.tensor.matmul(
                                st[:, ssl],
                                kT_sb[rsl, k0:k0 + P],
                                qT_sb[rsl, q0 + qv:q0 + QCH],
                                start=True, stop=(i < 0),
                            )
                            if i >= 0:
                                masks.append(half * QCH + qv)
                        # mask matmuls after both S halves: ident stays
                        # loaded as the stationary across consecutive ones
                        for m0 in masks:
                            nc.tensor.matmul(
                                st[:, m0:m0 + P], ident[:], mneg[:],
                                start=False, stop=True,
                            )
                        qv0 = max(2 * pr - 4 * jq, 0) * P
                        nc.scalar.activation(
                            pt[:, qv0:], st[:, qv0:], AF.Exp,
                            scale=float(SCALE)
                        )
                    return f

                def mk_pv(pr):
                    def f():
                        pt = cells[pr]["pt"]
                        yt = hstate["yt"]
                        for half in range(2):
                            kt = 2 * pr + half
                            qv = max(kt - 4 * jq, 0) * P
                            nc.tensor.matmul(
                                yt[:, qv:QCH],
                                v_sb[:, b * NKT + kt,
                                     h * (D + 1):(h + 1) * (D + 1)],
                                pt[:, half * QCH + qv:(half + 1) * QCH],
                                start=(kt == 0), stop=(kt == nkt - 1),
                            )
                        if pr == nkt // 2 - 1:
                            nc.vector.tensor_copy(hstate["den"][:],
                                                  yt[D:D + 1, :])
                    return f

                last = nkt // 2 - 1
                for pr in range(nkt // 2):
                    qv0 = max(2 * pr - 4 * jq, 0) * P
                    qv1 = max(2 * pr + 1 - 4 * jq, 0) * P
                    real = 2 * ((QCH - qv0) + (QCH - qv1))
                    real += 128 * ((2 * pr >= 4 * jq) + (2 * pr + 1 >= 4 * jq))
                    # exp period in PE-clock cycles: ACT runs at half the
                    # PE clock; ~490 covers the 352-cycle fixed cost plus
                    # observed PSUM-read contention
                    act = 2 * (2 * QCH - qv0 + 490)
                    emit_unit(
                        mk_s(pr), mk_pv(pr),
                        mk_evict(b, jq, h, hstate, then) if pr == last
                        else None,
                        budget=act - real,
                    )

            def a2a_fire(b):
                nc.gpsimd.collective_compute(
                    "AllToAll",
                    mybir.AluOpType.bypass,
                    replica_groups=[list(range(N_CORES))],
                    ins=[ain[b][:]],
                    outs=[aout[b][:]],
                )

            yf_tiles = {}

            def yf_load(b):
                yf = sb.tile([P, NCH, TSH], BF, tag="yf", bufs=2,
                             name="yf")
                yf_tiles[b] = yf
                nc.sync.dma_start(
                    yf[:, :, :],
                    aout[b][:].rearrange("a p t -> p a t"),
                )

            def po_group(b, store_engine):
                yf = yf_tiles[b]
                for om in range(NCH):
                    po = ps.tile([P, TSH], F32, tag="fx", bufs=2,
                                 name="po")
                    for ci in range(NCH):
                        nc.tensor.matmul(
                            po[:], wo_sb[:, om, ci, :], yf[:, ci, :],
                            start=(ci == 0), stop=(ci == NCH - 1),
                        )
                    ob = sb.tile([P, TSH], BF, tag="ob", bufs=2, name="ob")
                    nc.vector.tensor_copy(ob[:], po[:])
                    store_engine.dma_start(out[:, om, b, :], ob[:])

            # ---- prologue: warmups cover the first x DMAs ----
            warm(8)
            # tch0 projections as a dense block, then attention starts
            filler_q.extend(make_proj_fillers(0))
            flush_fillers()

            # filler deadline map: during (b, jq) we drip-feed the chunk
            # needed by the NEXT q-chunk phase, flushing at the boundary
            next_tch = {(0, 0): 1, (0, 1): 2, (0, 2): 3, (0, 3): 4,
                        (1, 0): 5, (1, 1): 6, (1, 2): 7}

            def warm_filler():
                wdst = ps.tile([P, QCH], F32, tag="fx", bufs=2, name="wdst")
                nc.tensor.matmul(wdst[:], ident[:], wsrc[:],
                                 start=True, stop=True)

            for b in range(B):
                for jq in range(NQC):
                    if b == 1 and jq == 0:
                        # cover the batch-boundary eviction flush so the
                        # PE stays dense while the vector chain drains
                        filler_q.extend([(512, warm_filler)] * 6)
                    if (b, jq) in next_tch:
                        filler_q.extend(make_proj_fillers(next_tch[(b, jq)]))
                    last_head = (jq == NQC - 1)
                    for h in range(HL):
                        # fire the batch's collective right after its last
                        # eviction DMAs; yf(1) SBUF loads are deferred to
                        # the tail so their DMA-lane sem increments don't
                        # precede po(0)'s stores in the scheduler's lane
                        # bookkeeping (po(0) would transitively wait on
                        # the second AllToAll)
                        then = None
                        if last_head and h == HL - 1:
                            if b == 0:
                                then = lambda: (a2a_fire(0), yf_load(0))
                            else:
                                then = lambda: a2a_fire(1)
                        emit_head(b, jq, h, then)
                    flush_fillers()
            # the final PV + evict + A2A(b1) are still pending here
            flush_pending()

            # ---- tail: po(b0) covers A2A(b1) latency; pinned late so the
            # scheduler cannot hoist the yf-gated matmuls into the
            # attention stream (they would stall the in-order PE queue)
            with tc.tile_wait_until(0.28):
                po_group(0, nc.scalar)
            with tc.tile_wait_until(0.285):
                yf_load(1)
                warm(24)
            with tc.tile_wait_until(0.3):
                po_group(1, nc.sync)

    nc.finalize()
    return nc


_GRAPH = None


def _get_graph():
    global _GRAPH
    if _GRAPH is None:
        _GRAPH = build_graph()
    return _GRAPH


def prepare_in_maps(x, Wq, Wk, Wv, Wo):
    x = np.asarray(x, np.float32)
    Wq = np.asarray(Wq, np.float32)
    Wk = np.asarray(Wk, np.float32)
    Wv = np.asarray(Wv, np.float32)
    Wo = np.asarray(Wo, np.float32)

    bf = ml_dtypes.bfloat16
    # [p, ci, t] with c = ci*128 + p
    xTh = np.ascontiguousarray(
        x.reshape(TT, NCH, P).transpose(2, 1, 0).reshape(P, NCH * TT)
    ).astype(bf)
    # full Wo packed [p, om, ci, m]: wo[p, om, ci, m] = Wo[om*128+m,
    # ci*128+p] (shared by all cores)
    woall = Wo.T.reshape(NCH, P, NCH, P).transpose(1, 2, 0, 3)
    woall = np.ascontiguousarray(woall.reshape(P, WO)).astype(bf)
    in_maps = []
    for r in range(N_CORES):
        sl = slice(r * DL, (r + 1) * DL)
        # pack the 3 transposed q/k/v shards into the SBUF layout
        # [p, w, ci, m] where the shard row index is c = ci*128 + p
        wqkv = np.empty((P, 3, NCH, DL), np.float32)
        for w, W in enumerate((Wq, Wk, Wv)):
            wqkv[:, w] = W[sl].T.reshape(NCH, P, DL).transpose(1, 0, 2)
        wall = np.concatenate(
            [np.ascontiguousarray(wqkv.reshape(P, WQKV)).astype(bf), woall],
            axis=1,
        )
        in_maps.append({
            "xT": xTh,
            "wall": np.ascontiguousarray(wall),
        })
    return in_maps


def assemble_output(results):
    outT = np.empty((B, C, T), np.float32)
    for r in range(N_CORES):
        o = np.asarray(results[r]["out"], np.float32)  # [m, om, b, t]
        # feature index = om*128 + m; token = r*256 + t within batch
        oT = o.transpose(2, 1, 0, 3).reshape(B, C, TSH)
        outT[:, :, r * TSH:(r + 1) * TSH] = oT
    return np.ascontiguousarray(outT.transpose(0, 2, 1))


def kernel(x, Wq, Wk, Wv, Wo):
    nc = _get_graph()
    in_maps = prepare_in_maps(x, Wq, Wk, Wv, Wo)
    res = run_bass_kernel_spmd(nc, in_maps, core_ids=list(range(N_CORES)))
    return assemble_output(res.results)


# revision 34
# speedup vs baseline: 1.4259x; 1.0850x over previous
"""Causal multi-head attention block (B=2, T=2048, C=1024, H=16) on 8 TRN2
NeuronCores.

Sharding: Megatron-style tensor parallel over heads for QKV+attention
(core r owns heads {2r, 2r+1} = feature rows [128r, 128r+128) of q/k/v),
then a token-sharded output projection: instead of AllGathering y
(7 MB inbound wire per core at ~60 GB/s bus ~= 117 us of CC-stream time),
the cores AllToAll y so core r ends up with y_full[:, its token slice]
(0.875 MB wire per core, ~13 us) and computes ALL 1024 output features
for its 256-token slice of each batch entry with the full Wo (same FLOPs
as the feature-sharded O-projection: contract is 1024 either way).

On-device everything is computed feature-major (transposed):

  qT/kT/vT [128, 4096] = W_shard @ x^T          (x^T passed from host)
  ST tile [128k, 512q] = kT_slice.T @ qT_slice  (contract d=64)
  causal mask: add a -1e9 strictly-lower-triangular matrix into the St
      PSUM accumulation group via matmul(ident, mneg) on diagonal blocks
  PT = exp(ST * 1/sqrt(d))                      (no max-subtraction: logits
                                                 are ~N(0,1), |S|max ~ 6)
  yT [65, 512] += [v | 1].T @ PT                (row 64 = softmax denom)
  yT_norm = yT[0:64] * partition_broadcast(recip(yT[64]))
  AllToAll per batch entry: blocks of [128 local feats, 256 tokens]
  po [128 out-feats, 256 tok] per om-chunk = Wo_full @ y_full_slice

Performance structure (derived from perfetto/HAM analysis; the AllGather
baseline ran 305 us, this runs ~230 us true path):
- The PE HAM clock-gate is a proportional governor (K tracks array
  activity), so the schedule keeps the PE queue saturated: attention
  pairs are emitted with a 1-deep software pipeline skew (S(p+1) is
  program-ordered before PV(p), covering the exp(p) latency with real
  work), and each pair unit pops projection micro-fillers (2-matmul
  closures) sized by a cycle budget = exp-period minus the unit's own
  S/PV work — the PE stays dense without delaying the ACT-bound exp
  cadence; dummy 512-col matmuls fill when no real filler remains.
- Attention starts as soon as tch0 is projected; x arrives via one
  contiguous DMA per token chunk ([p, ci, t] host layout) on the gpsimd
  queue so the scalar queue is free for the exps (a DMA issue occupies
  its issuing engine queue ~0.7 us).
- The two AllToAlls fire per batch as soon as that batch's heads are
  evicted; the CC entry barrier (launch-skew rendezvous) overlaps
  compute. DMA-completion semaphore lanes are shared round-robin across
  queues, so yf(1)'s loads are program-ordered AFTER po(0)'s stores —
  otherwise po(0)'s release thresholds would transitively wait on the
  second collective.
- O-projections run at the tail, pinned there with tile_wait_until so
  the scheduler cannot hoist the collective-gated matmuls into the
  attention stream (the in-order PE queue would stall); po(b0) covers
  the A2A(b1) latency, keep-warm matmuls bridge the remaining gap.

Inputs are bf16 (host-side cast); accumulation is f32 in PSUM; the
output shard is written bf16 and upcast to f32 on the host.
"""

import numpy as np
import ml_dtypes

import concourse.bacc as bacc
import concourse.mybir as mybir
import concourse.tile as tile
from concourse.bass_utils import run_bass_kernel_spmd
from concourse.masks import make_identity

N_CORES = 8
B, T, C, H = 2, 2048, 1024, 16
D = 64                # head dim
HL = H // N_CORES     # heads per core = 2
DL = HL * D           # local feature dim = 128
TT = B * T            # 4096 tokens total
P = 128
NCH = C // P          # 8 contraction chunks
QCH = 512             # projection token-chunk (moving free dim)
NQC = T // QCH        # 4 projection chunks per batch entry
WQ = 1024             # attention q-window (bf16 moving-operand max)
NWQ = T // WQ         # 2 q-windows per batch entry
NKT = T // P          # 16 k-tiles per batch entry
TSH = T // N_CORES    # 256: tokens per core per batch after AllToAll
SCALE = 1.0 / np.sqrt(D)

BF = mybir.dt.bfloat16
F32 = mybir.dt.float32
AF = mybir.ActivationFunctionType

WQKV = 3 * NCH * DL           # 3072 cols of packed q/k/v shards
WO = NCH * NCH * P            # 8192 cols of packed full Wo
WCOLS = WQKV + WO             # 11264


def build_graph():
    nc = bacc.Bacc("TRN2", target_bir_lowering=False, debug=False)

    # [p, ci, t] with c = ci*128 + p: one contiguous DMA per token chunk
    xT = nc.dram_tensor("xT", [P, NCH * TT], BF, kind="ExternalInput")
    # q/k/v shards [p, w, ci, m] + full Wo [p, om, ci, m], pre-packed
    # host-side into SBUF layout: contiguous rows, loaded as 8 DMAs
    wall = nc.dram_tensor("wall", [P, WCOLS], BF, kind="ExternalInput")
    # (m, om, b, t): feature om*128+m, batch b, token-in-shard t
    out = nc.dram_tensor("out", [P, NCH, B, TSH], BF, kind="ExternalOutput")

    with tile.TileContext(nc) as tc:
        with (
            tc.tile_pool(name="sb", bufs=1) as sb,
            tc.tile_pool(name="ps", bufs=1, space="PSUM") as ps,
            tc.tile_pool(name="dram", bufs=1, space="DRAM") as dram,
        ):
            # ---- weight + constant loads ----
            w_sb = sb.tile([P, WCOLS], BF, name="w_sb")
            for pc in range(8):
                csl = slice(pc * (WCOLS // 8), (pc + 1) * (WCOLS // 8))
                nc.sync.dma_start(w_sb[:, csl], wall[:, csl])
            w3 = w_sb[:, 0:WQKV].rearrange("p (w a m) -> p w a m", w=3, a=NCH)
            wq_sb, wk_sb, wv_sb = (w3[:, i] for i in range(3))
            wo_sb = w_sb[:, WQKV:WCOLS].rearrange(
                "p (o a m) -> p o a m", o=NCH, a=NCH
            )

            ident = sb.tile([P, P], BF, name="ident")
            make_identity(nc, ident)
            wsrc = sb.tile([P, QCH], BF, name="wsrc")
            nc.vector.memset(wsrc[:], 0.5)

            xT_sb = sb.tile([P, NCH, TT], BF, name="xT_sb")
            xTv = xT[:].rearrange("p (a t) -> p a t", a=NCH)
            # x lands in tch order just ahead of each chunk's projection;
            # gpsimd (SWDGE) so the scalar queue stays free for the exps
            # and sync for the weights
            nc.gpsimd.dma_start(xT_sb[:, :, 0:QCH], xTv[:, :, 0:QCH])
            nc.gpsimd.dma_start(xT_sb[:, :, QCH:2 * QCH],
                                xTv[:, :, QCH:2 * QCH])

            # strictly-lower-triangular -1e9 (k > q): masks causal logits on
            # diagonal blocks, injected into the St PSUM group via
            # matmul(ident, mneg)
            mneg = sb.tile([P, P], BF, name="mneg")
            nc.gpsimd.memset(mneg[:], 0.0)
            nc.gpsimd.affine_select(
                out=mneg[:], in_=mneg[:],
                compare_op=mybir.AluOpType.is_ge,
                fill=-1e9, base=0, channel_multiplier=-1, pattern=[[1, P]],
            )

            def warm(n):
                for _ in range(n):
                    wdst = ps.tile([P, QCH], F32, tag="fx", bufs=2,
                                   name="wdst")
                    nc.tensor.matmul(wdst[:], ident[:], wsrc[:],
                                     start=True, stop=True)

            qT_sb = sb.tile([P, TT], BF, name="qT_sb")
            kT_sb = sb.tile([P, TT], BF, name="kT_sb")
            vT_sb = sb.tile([P, TT], BF, name="vT_sb")
            # v in natural layout, packed per 128-token tile as
            # [headA(64) | 1 | headB(64) | 1] -> 130 columns
            v_sb = sb.tile([P, TT // P, 2 * (D + 1)], BF, name="v_sb")
            nc.gpsimd.memset(v_sb[:], 1.0)
            # remaining x chunks after the memset (needed later anyway)
            for tch in range(2, 8):
                tsl = slice(tch * QCH, (tch + 1) * QCH)
                nc.gpsimd.dma_start(xT_sb[:, :, tsl], xTv[:, :, tsl])

            # AllToAll buffers: per batch, 8 blocks of [128 feats, 256 tok]
            ain = [dram.tile([N_CORES, DL, TSH], BF, name=f"ain{b}")
                   for b in range(B)]
            aout = [dram.tile([N_CORES, DL, TSH], BF, name=f"aout{b}")
                    for b in range(B)]

            # ---- projection micro-fillers (cost, closure) pairs; cost is
            # PE cycles so the attention pacer can fill each unit's slack
            def make_proj_fillers(tch):
                """2-matmul closures for q/k/v projection of token chunk
                tch + the 4 v-transposes."""
                tsl = slice(tch * QCH, (tch + 1) * QCH)
                fillers = []
                for wsb, dst in ((wq_sb, qT_sb), (wk_sb, kT_sb),
                                 (wv_sb, vT_sb)):
                    cell = {}
                    def mk(ci0, wsb=wsb, dst=dst, cell=cell):
                        def f():
                            if ci0 == 0:
                                cell["pj"] = ps.tile([P, QCH], F32,
                                                     tag="fx", bufs=2,
                                                     name="pj")
                            pj = cell["pj"]
                            for ci in (ci0, ci0 + 1):
                                nc.tensor.matmul(
                                    pj[:], wsb[:, ci, :], xT_sb[:, ci, tsl],
                                    start=(ci == 0), stop=(ci == NCH - 1),
                                )
                            if ci0 == NCH - 2:
                                nc.vector.tensor_copy(dst[:, tsl], pj[:])
                        return f
                    for ci0 in range(0, NCH, 2):
                        fillers.append((1024, mk(ci0)))
                for t32 in range(tch * 4, tch * 4 + 4):
                    def vt(t32=t32):
                        tr = ps.tile([P, P], BF, tag="fx", bufs=2,
                                     name="tr")
                        nc.tensor.transpose(
                            tr[:], vT_sb[:, t32 * P:(t32 + 1) * P], ident[:]
                        )
                        out_ap = v_sb[:, t32, :].rearrange(
                            "p (h x) -> p h x", h=HL
                        )[:, :, 0:D]
                        in_ap = tr[:].rearrange("p (h x) -> p h x", h=HL)
                        nc.vector.tensor_copy(out_ap, in_ap)
                    fillers.append((128, vt))
                return fillers

            # ---- attention emission machinery ----
            filler_q = []
            pending = []   # at most one (pv_fn, post_fn)

            def flush_pending():
                while pending:
                    pv, post = pending.pop(0)
                    pv()
                    if post:
                        post()

            def emit_unit(s_fn, pv_fn=None, post_fn=None, budget=0):
                """budget: PE cycles of filler to interleave so the unit's
                total PE work matches the ACT (exp) period — keeps the PE
                saturated (HAM at full clock) without delaying the exps."""
                s_fn()
                while budget > 0:
                    if filler_q:
                        cost, f = filler_q.pop(0)
                        f()
                        budget -= cost
                    else:
                        # dummy matmul: keeps the PE above the HAM
                        # activity threshold when no real filler remains
                        warm_filler()
                        budget -= 512
                if pending:
                    pv, post = pending.pop(0)
                    pv()
                    if post:
                        post()
                if pv_fn:
                    pending.append((pv_fn, post_fn))

            def flush_fillers():
                while filler_q:
                    filler_q.pop(0)[1]()

            def mk_evict(b, jq, h, hstate, then=None):
                def f():
                    yt, den = hstate["yt"], hstate["den"]
                    bc = sb.tile([D, QCH], F32, tag="bc", bufs=3, name="bc")
                    nc.gpsimd.partition_broadcast(bc[:], den[:])
                    rcp = sb.tile([D, QCH], F32, tag="rcp", bufs=3,
                                  name="rcp")
                    nc.vector.reciprocal_approx_fast(rcp[:], bc[:])
                    yn = sb.tile([D, QCH], BF, tag="yn", bufs=4, name="yn")
                    nc.vector.tensor_mul(yn[:], yt[0:D, :], rcp[:])
                    for s in range(2):
                        nc.gpsimd.dma_start(
                            ain[b][2 * jq + s, h * D:(h + 1) * D, :],
                            yn[:, s * TSH:(s + 1) * TSH],
                        )
                    if then:
                        then()
                return f

            def emit_head(b, jq, h, then=None):
                rsl = slice(h * D, (h + 1) * D)
                q0 = b * T + jq * QCH
                nkt = 4 * jq + 4
                hstate = {}
                cells = [dict() for _ in range(nkt // 2)]

                def mk_s(pr):
                    def f():
                        if pr == 0:
                            hstate["yt"] = ps.tile([D + 1, QCH], F32,
                                                   tag="yt", bufs=2,
                                                   name="yt")
                            hstate["den"] = sb.tile([1, QCH], F32,
                                                    tag="den", bufs=4,
                                                    name="den")
                        st = ps.tile([P, 2 * QCH], F32, tag="st", bufs=2,
                                     name="st")
                        pt = sb.tile([P, 2 * QCH], BF, tag="pt", bufs=4,
                                     name="pt")
                        cells[pr]["pt"] = pt
                        masks = []
                        for half in range(2):
                            kt = 2 * pr + half
                            k0 = b * T + kt * P
                            i = kt - 4 * jq
                            qv = max(i, 0) * P
                            ssl = slice(half * QCH + qv, (half + 1) * QCH)
                            nc.tensor.matmul(
                                st[:, ssl],
                                kT_sb[rsl, k0:k0 + P],
                                qT_sb[rsl, q0 + qv:q0 + QCH],
                                start=True, stop=(i < 0),
                            )
                            if i >= 0:
                                masks.append(half * QCH + qv)
                        # mask matmuls after both S halves: ident stays
                        # loaded as the stationary across consecutive ones
                        for m0 in masks:
                            nc.tensor.matmul(
                                st[:, m0:m0 + P], ident[:], mneg[:],
                                start=False, stop=True,
                            )
                        qv0 = max(2 * pr - 4 * jq, 0) * P
                        nc.scalar.activation(
                            pt[:, qv0:], st[:, qv0:], AF.Exp,
                            scale=float(SCALE)
                        )
                    return f

                def mk_pv(pr):
                    def f():
                        pt = cells[pr]["pt"]
                        yt = hstate["yt"]
                        for half in range(2):
                            kt = 2 * pr + half
                            qv = max(kt - 4 * jq, 0) * P
                            nc.tensor.matmul(
                                yt[:, qv:QCH],
                                v_sb[:, b * NKT + kt,
                                     h * (D + 1):(h + 1) * (D + 1)],
                                pt[:, half * QCH + qv:(half + 1) * QCH],
                                start=(kt == 0), stop=(kt == nkt - 1),
                            )
                        if pr == nkt // 2 - 1:
                            nc.vector.tensor_copy(hstate["den"][:],
                                                  yt[D:D + 1, :])
                    return f

                last = nkt // 2 - 1
                for pr in range(nkt // 2):
                    qv0 = max(2 * pr - 4 * jq, 0) * P
                    qv1 = max(2 * pr + 1 - 4 * jq, 0) * P
                    real = 2 * ((QCH - qv0) + (QCH - qv1))
                    real += 128 * ((2 * pr >= 4 * jq) + (2 * pr + 1 >= 4 * jq))
                    # exp period in PE-clock cycles: ACT runs at half the
                    # PE clock; ~280 is the measured effective per-exp
                    # fixed cost (1070 ns for a full 1024-col exp)
                    act = 2 * (2 * QCH - qv0 + 280)
                    emit_unit(
                        mk_s(pr), mk_pv(pr),
                        mk_evict(b, jq, h, hstate, then) if pr == last
                        else None,
                        budget=act - real,
                    )

            def a2a_fire(b):
                nc.gpsimd.collective_compute(
                    "AllToAll",
                    mybir.AluOpType.bypass,
                    replica_groups=[list(range(N_CORES))],
                    ins=[ain[b][:]],
                    outs=[aout[b][:]],
                )

            yf_tiles = {}

            def yf_load(b):
                yf = sb.tile([P, NCH, TSH], BF, tag="yf", bufs=2,
                             name="yf")
                yf_tiles[b] = yf
                nc.sync.dma_start(
                    yf[:, :, :],
                    aout[b][:].rearrange("a p t -> p a t"),
                )

            def po_group(b, store_engine):
                yf = yf_tiles[b]
                for om in range(NCH):
                    po = ps.tile([P, TSH], F32, tag="fx", bufs=2,
                                 name="po")
                    for ci in range(NCH):
                        nc.tensor.matmul(
                            po[:], wo_sb[:, om, ci, :], yf[:, ci, :],
                            start=(ci == 0), stop=(ci == NCH - 1),
                        )
                    ob = sb.tile([P, TSH], BF, tag="ob", bufs=2, name="ob")
                    nc.vector.tensor_copy(ob[:], po[:])
                    store_engine.dma_start(out[:, om, b, :], ob[:])

            # ---- prologue: warmups cover the first x DMAs ----
            warm(8)
            # tch0 projections as a dense block, then attention starts
            filler_q.extend(make_proj_fillers(0))
            flush_fillers()

            # filler deadline map: during (b, jq) we drip-feed the chunk
            # needed by the NEXT q-chunk phase, flushing at the boundary
            next_tch = {(0, 0): 1, (0, 1): 2, (0, 2): 3, (0, 3): 4,
                        (1, 0): 5, (1, 1): 6, (1, 2): 7}

            def warm_filler():
                wdst = ps.tile([P, QCH], F32, tag="fx", bufs=2, name="wdst")
                nc.tensor.matmul(wdst[:], ident[:], wsrc[:],
                                 start=True, stop=True)

            for b in range(B):
                for jq in range(NQC):
                    if b == 1 and jq == 0:
                        # cover the batch-boundary eviction flush so the
                        # PE stays dense while the vector chain drains
                        filler_q.extend([(512, warm_filler)] * 6)
                    if (b, jq) in next_tch:
                        filler_q.extend(make_proj_fillers(next_tch[(b, jq)]))
                    last_head = (jq == NQC - 1)
                    for h in range(HL):
                        # fire the batch's collective right after its last
                        # eviction DMAs; yf(1) SBUF loads are deferred to
                        # the tail so their DMA-lane sem increments don't
                        # precede po(0)'s stores in the scheduler's lane
                        # bookkeeping (po(0) would transitively wait on
                        # the second AllToAll)
                        then = None
                        if last_head and h == HL - 1:
                            if b == 0:
                                then = lambda: (a2a_fire(0), yf_load(0))
                            else:
                                then = lambda: a2a_fire(1)
                        emit_head(b, jq, h, then)
                    flush_fillers()
            # the final PV + evict + A2A(b1) are still pending here
            flush_pending()

            # ---- tail: po(b0) covers A2A(b1) latency; pinned late so the
            # scheduler cannot hoist the yf-gated matmuls into the
            # attention stream (they would stall the in-order PE queue)
            with tc.tile_wait_until(0.28):
                po_group(0, nc.scalar)
            with tc.tile_wait_until(0.285):
                yf_load(1)
                warm(24)
            with tc.tile_wait_until(0.3):
                po_group(1, nc.sync)

    nc.finalize()
    return nc


_GRAPH = None


def _get_graph():
    global _GRAPH
    if _GRAPH is None:
        _GRAPH = build_graph()
    return _GRAPH


def prepare_in_maps(x, Wq, Wk, Wv, Wo):
    x = np.asarray(x, np.float32)
    Wq = np.asarray(Wq, np.float32)
    Wk = np.asarray(Wk, np.float32)
    Wv = np.asarray(Wv, np.float32)
    Wo = np.asarray(Wo, np.float32)

    bf = ml_dtypes.bfloat16
    # [p, ci, t] with c = ci*128 + p
    xTh = np.ascontiguousarray(
        x.reshape(TT, NCH, P).transpose(2, 1, 0).reshape(P, NCH * TT)
    ).astype(bf)
    # full Wo packed [p, om, ci, m]: wo[p, om, ci, m] = Wo[om*128+m,
    # ci*128+p] (shared by all cores)
    woall = Wo.T.reshape(NCH, P, NCH, P).transpose(1, 2, 0, 3)
    woall = np.ascontiguousarray(woall.reshape(P, WO)).astype(bf)
    in_maps = []
    for r in range(N_CORES):
        sl = slice(r * DL, (r + 1) * DL)
        # pack the 3 transposed q/k/v shards into the SBUF layout
        # [p, w, ci, m] where the shard row index is c = ci*128 + p
        wqkv = np.empty((P, 3, NCH, DL), np.float32)
        for w, W in enumerate((Wq, Wk, Wv)):
            wqkv[:, w] = W[sl].T.reshape(NCH, P, DL).transpose(1, 0, 2)
        wall = np.concatenate(
            [np.ascontiguousarray(wqkv.reshape(P, WQKV)).astype(bf), woall],
            axis=1,
        )
        in_maps.append({
            "xT": xTh,
            "wall": np.ascontiguousarray(wall),
        })
    return in_maps


def assemble_output(results):
    outT = np.empty((B, C, T), np.float32)
    for r in range(N_CORES):
        o = np.asarray(results[r]["out"], np.float32)  # [m, om, b, t]
        # feature index = om*128 + m; token = r*256 + t within batch
        oT = o.transpose(2, 1, 0, 3).reshape(B, C, TSH)
        outT[:, :, r * TSH:(r + 1) * TSH] = oT
    return np.ascontiguousarray(outT.transpose(0, 2, 1))


def kernel(x, Wq, Wk, Wv, Wo):
    nc = _get_graph()
    in_maps = prepare_in_maps(x, Wq, Wk, Wv, Wo)
    res = run_bass_kernel_spmd(nc, in_maps, core_ids=list(range(N_CORES)))
    return assemble_output(res.results)
